# revision 14
# baseline (speedup 1.0000x reference)
"""ChaosAttention on 8 Trainium2 NeuronCores.

Sharding: tensor-parallel over heads. Each of the 8 cores owns H/8 = 2 heads
(128 of the 1024 q/k/v projection columns, 128 of the Wo rows). Every core
reads the full x (as x^T, bf16); the out-projection is row-parallel, so each
core returns a partial y^T and the host sums the 8 partials and adds bo.

The physics adapter (physics_state @ Wp1 -> gelu -> @ Wp2) produces a bias
that is constant along the softmax axis (it is broadcast over both the query
and key dims of the scores). softmax(x + c) == softmax(x), and masked
positions are -inf either way, so the adapter has zero effect on the output
and is skipped entirely.

Key compaction: masked keys contribute exactly zero attention weight (their
scores are -inf in the reference), so the host packs only the kept keys per
batch (padded to a multiple of 128; pad slots get a -1e30 exp bias). With a
~50% random mask this halves the QK^T / exp / AV work with identical math.

Device kernel layout (per core):
  - scores are computed transposed (S^T: keys on partitions, queries on the
    free dim) so the pad-mask is a per-partition bias fused into the
    ACT-engine exp, and the AV matmul needs no transpose of the probs.
  - the two heads are row-packed into the PE array for QK^T (head 0 in rows
    0-63, head 1 in rows 64-127 via tile_position): full K=128 utilisation.
  - V is kept in natural layout with an appended ones-column, so the AV
    matmul (V' stationary, M=65) yields both the unnormalised output and the
    softmax denominator Z in one PSUM accumulation.
  - 1/Z: one batched DVE reciprocal per (b, ic) covering both heads, then
    gpsimd.partition_broadcast and one DVE multiply; softmax max-subtraction
    is skipped (scores are O(1), exp cannot overflow in fp32).
  - q/k/v/AT chunks live in separate 512-token tiles so the Tile scheduler's
    dependency tracking lets attention start while projections still run.
"""

import numpy as np
import ml_dtypes

_BF16 = ml_dtypes.bfloat16

B, T, E, H, D = 2, 2048, 1024, 16, 64
BT = B * T                 # 4096 tokens
N_CORES = 8
PCN = E // N_CORES         # 128 per-core projection dims (2 heads x 64)
EC = E // 128              # 8 contraction chunks for the projections
IC = 512                   # query-chunk size
NQC = T // IC              # 4 query chunks per batch
SCALE = 1.0 / float(np.sqrt(D))

_cache = {}


def _build(nkt):
    """Build + schedule the per-core Bass program. nkt = packed key-tile
    counts per batch (ceil(kept/128))."""
    key = tuple(nkt)
    if key in _cache:
        return _cache[key]

    from contextlib import ExitStack
    import concourse.tile as tile
    from concourse import bacc, mybir

    f32 = mybir.dt.float32
    bf16 = mybir.dt.bfloat16
    Exp = mybir.ActivationFunctionType.Exp

    base = [0, nkt[0]]              # packed key-tile offset per batch
    ntt = nkt[0] + nkt[1]           # total packed key tiles
    KP = ntt * 128                  # total packed+padded kv tokens

    nc = bacc.Bacc("TRN2", target_bir_lowering=False, debug=False,
                   num_devices=N_CORES)

    xT_d = nc.dram_tensor("xT", [E, BT], bf16, kind="ExternalInput").ap()
    xk_d = nc.dram_tensor("xk", [E, KP], bf16, kind="ExternalInput").ap()
    wq_d = nc.dram_tensor("wq", [E, PCN], bf16, kind="ExternalInput").ap()
    wk_d = nc.dram_tensor("wk", [E, PCN], bf16, kind="ExternalInput").ap()
    wv_d = nc.dram_tensor("wv", [E, PCN], bf16, kind="ExternalInput").ap()
    wo_d = nc.dram_tensor("wo", [PCN, E], bf16, kind="ExternalInput").ap()
    bq_d = nc.dram_tensor("bq", [PCN, 1], f32, kind="ExternalInput").ap()
    bk_d = nc.dram_tensor("bk", [PCN, 1], f32, kind="ExternalInput").ap()
    bv_d = nc.dram_tensor("bv", [PCN, 1], f32, kind="ExternalInput").ap()
    mb_d = nc.dram_tensor("mb", [128, ntt], f32, kind="ExternalInput").ap()
    yT_d = nc.dram_tensor("yT", [E, BT], f32, kind="ExternalOutput").ap()

    def kwins(b):
        """(start, size) 512-col windows over batch b's packed kv columns."""
        n = nkt[b] * 128
        return [(w * 512, min(512, n - w * 512)) for w in range((n + 511) // 512)]

    with tile.TileContext(nc) as tc, ExitStack() as ctx:
        consts = ctx.enter_context(tc.tile_pool(name="consts", bufs=1))
        pp_mm = ctx.enter_context(tc.tile_pool(name="ppmm", bufs=2, space="PSUM"))
        pp_st = ctx.enter_context(tc.tile_pool(name="ppst", bufs=2, space="PSUM"))
        pp_o = ctx.enter_context(tc.tile_pool(name="ppo", bufs=2, space="PSUM"))
        pool_pt = ctx.enter_context(tc.tile_pool(name="ptp", bufs=4))
        pool_oc = ctx.enter_context(tc.tile_pool(name="ocp", bufs=2))
        pool_rz = ctx.enter_context(tc.tile_pool(name="rzp", bufs=2))
        pool_zz = ctx.enter_context(tc.tile_pool(name="zzp", bufs=2))
        pool_rb = ctx.enter_context(tc.tile_pool(name="rbp", bufs=2))
        pool_y = ctx.enter_context(tc.tile_pool(name="yp", bufs=6))

        # ---- persistent SBUF residents ----
        # x^T for q in 1024-token windows; packed kv x^T in 512-col windows.
        xq = [consts.tile([128, EC, 512], bf16, tag=f"xq{mw}", name=f"xq{mw}")
              for mw in range(8)]

        def xq_sl(ec, m0, sz):
            assert sz == 512 and m0 % 512 == 0
            return xq[m0 // 512][:, ec, :]

        xkw = {}
        for b in range(B):
            for wi, (w0, wsz) in enumerate(kwins(b)):
                xkw[(b, wi)] = consts.tile([128, EC, wsz], bf16,
                                           tag=f"xk{b}_{wi}", name=f"xk{b}_{wi}")

        wq_sb = consts.tile([128, EC, PCN], bf16, tag="wq")
        wk_sb = consts.tile([128, EC, PCN], bf16, tag="wk")
        wv_sb = consts.tile([128, EC, PCN], bf16, tag="wv")
        for w_sb, w_d in ((wq_sb, wq_d), (wk_sb, wk_d), (wv_sb, wv_d)):
            nc.sync.dma_start(w_sb[:], w_d.rearrange("(c p) n -> p c n", p=128))
        wo_sb = consts.tile([128, E], bf16, tag="wo")
        nc.sync.dma_start(wo_sb[:], wo_d[:])
        mb_sb = consts.tile([128, ntt], f32, tag="mb")
        nc.sync.dma_start(mb_sb[:], mb_d[:])
        bq_sb = consts.tile([128, 1], f32, tag="bq")
        bk_sb = consts.tile([128, 1], f32, tag="bk")
        bv_sb = consts.tile([128, 1], f32, tag="bv")
        for b_sb, b_d in ((bq_sb, bq_d), (bk_sb, bk_d), (bv_sb, bv_d)):
            nc.sync.dma_start(b_sb[:], b_d[:])

        def dma_xk(b, wi):
            w0, wsz = kwins(b)[wi]
            c0 = base[b] * 128 + w0
            for ec in range(EC):
                nc.sync.dma_start(xkw[(b, wi)][:, ec, :],
                                  xk_d[ec * 128:(ec + 1) * 128, c0:c0 + wsz])

        def dma_xq(mw):
            for ec in range(EC):
                nc.sync.dma_start(
                    xq[mw][:, ec, :],
                    xT_d[ec * 128:(ec + 1) * 128, mw * 512:(mw + 1) * 512])

        # b0 kv windows and b0 q windows interleaved so the b0 projections
        # (and with them the first attention chunk) unblock earliest.
        dma_xk(0, 0)
        dma_xq(0)
        for wi in range(1, len(kwins(0))):
            dma_xk(0, wi)
        for mw in range(1, 4):
            dma_xq(mw)
        for wi in range(len(kwins(1))):
            dma_xk(1, wi)
        for mw in range(4, 8):
            dma_xq(mw)

        # per-chunk tiles: fine-grained deps let attention start early
        qTc = {(b, icx): consts.tile([128, IC], bf16, tag=f"qT{b}_{icx}",
                                     name=f"qT{b}_{icx}")
               for b in range(B) for icx in range(NQC)}
        ATc = {(b, icx): consts.tile([128, IC], bf16, tag=f"AT{b}_{icx}",
                                     name=f"AT{b}_{icx}")
               for b in range(B) for icx in range(NQC)}
        kTc = {}
        Vpg = {}
        for b in range(B):
            for wi, (w0, wsz) in enumerate(kwins(b)):
                kTc[(b, wi)] = consts.tile([128, wsz], bf16, tag=f"kT{b}_{wi}",
                                           name=f"kT{b}_{wi}")
                Vpg[(b, wi)] = consts.tile([128, wsz // 128, 2, D + 1], bf16,
                                           tag=f"Vp{b}_{wi}", name=f"Vp{b}_{wi}")
                nc.gpsimd.memset(Vpg[(b, wi)][:, :, :, D:D + 1], 1.0)

        # ---- phase emitters ----
        def proj_q_chunk(b, icx):
            m0 = b * T + icx * IC
            ps = pp_mm.tile([128, 512], f32, tag="mm", name="psq")
            for ec in range(EC):
                nc.tensor.matmul(ps[:], lhsT=wq_sb[:, ec, :],
                                 rhs=xq_sl(ec, m0, IC),
                                 start=(ec == 0), stop=(ec == EC - 1))
            nc.vector.tensor_scalar_add(out=qTc[(b, icx)][:], in0=ps[:],
                                        scalar1=bq_sb[:])

        def proj_k_chunk(b, wi, wsz):
            ps = pp_mm.tile([128, 512], f32, tag="mm", name="psk")
            for ec in range(EC):
                nc.tensor.matmul(ps[:, 0:wsz], lhsT=wk_sb[:, ec, :],
                                 rhs=xkw[(b, wi)][:, ec, :],
                                 start=(ec == 0), stop=(ec == EC - 1))
            nc.vector.tensor_scalar_add(out=kTc[(b, wi)][:], in0=ps[:, 0:wsz],
                                        scalar1=bk_sb[:])

        def proj_v_group(b, wi, wsz):
            nt = wsz // 128
            ps = pp_mm.tile([128, 512], f32, tag="mm", name="psv")
            for q in range(nt):
                for ec in range(EC):
                    nc.tensor.matmul(ps[:, q * 128:(q + 1) * 128],
                                     lhsT=xkw[(b, wi)][:, ec, q * 128:(q + 1) * 128],
                                     rhs=wv_sb[:, ec, :],
                                     start=(ec == 0), stop=(ec == EC - 1))
            nc.vector.tensor_copy(
                out=Vpg[(b, wi)][:, :, :, 0:D],
                in_=ps[:, 0:wsz].rearrange("p (q h d) -> p q h d", q=nt, h=2))

        def proj_items(b):
            items = []
            wins = kwins(b)
            for wi, (w0, wsz) in enumerate(wins):
                items.append(lambda b=b, wi=wi, wsz=wsz: proj_k_chunk(b, wi, wsz))
                items.append(lambda b=b, wi=wi, wsz=wsz: proj_v_group(b, wi, wsz))
            items.insert(2, lambda b=b: proj_q_chunk(b, 0))
            for icx in range(1, NQC):
                items.append(lambda b=b, icx=icx: proj_q_chunk(b, icx))
            return items

        def outproj_unit(b, icx, et):
            yp = pp_mm.tile([128, 512], f32, tag="mm", name="psy")
            nc.tensor.matmul(yp[:], lhsT=wo_sb[:, et * 128:(et + 1) * 128],
                             rhs=ATc[(b, icx)][:], start=True, stop=True)
            ysb = pool_y.tile([128, 512], f32, tag="y")
            # all staging copies on DVE: ACT's in-order queue stays pure
            # exps, which is the boundary-critical chain
            nc.vector.tensor_copy(out=ysb[:], in_=yp[:])
            i0 = b * T + icx * IC
            nc.sync.dma_start(yT_d[et * 128:(et + 1) * 128, i0:i0 + 512], ysb[:])

        def outproj_units(b, icx):
            return [lambda et=et: outproj_unit(b, icx, et) for et in range(EC)]

        def attn_chunk(b, icx, light, heavy, norm_prev):
            i0 = b * T + icx * IC
            o_ps = [pp_o.tile([D + 1, IC], f32, tag="o", name=f"o{h}")
                    for h in range(2)]
            for jt in range(nkt[b]):
                tg = base[b] + jt
                wi, q = jt // 4, jt % 4
                st = pp_st.tile([128, 2 * IC], f32, tag="st")
                nc.tensor.matmul(st[:, 0:IC],
                                 lhsT=kTc[(b, wi)][0:64, q * 128:(q + 1) * 128],
                                 rhs=qTc[(b, icx)][0:64, :],
                                 tile_position=(0, 0), start=True, stop=True)
                nc.tensor.matmul(st[:, IC:2 * IC],
                                 lhsT=kTc[(b, wi)][64:128, q * 128:(q + 1) * 128],
                                 rhs=qTc[(b, icx)][64:128, :],
                                 tile_position=(64, 0), start=True, stop=True)
                pt = pool_pt.tile([128, 2 * IC], bf16, tag="pt")
                nc.scalar.activation(out=pt[:], in_=st[:], func=Exp,
                                     bias=mb_sb[:, tg:tg + 1], scale=SCALE)
                for h in range(2):
                    nc.tensor.matmul(o_ps[h][:], lhsT=Vpg[(b, wi)][:, q, h, :],
                                     rhs=pt[:, h * IC:(h + 1) * IC],
                                     start=(jt == 0), stop=(jt == nkt[b] - 1))
                if light:
                    light.pop(0)()
                if norm_prev and jt >= 2:
                    norm_prev.pop(0)()
                if jt % 3 == 1 and heavy:
                    heavy.pop(0)()
            # free the o_ps slots fast: only the PSUM->SBUF copies happen at
            # the boundary; the reciprocal/broadcast/mul tail is deferred
            # into the next chunk (norm2), keeping ACT on back-to-back exps.
            ocs = []
            for h in range(2):
                oc = pool_oc.tile([D + 1, IC], f32, tag="oc", name=f"oc{h}")
                nc.vector.tensor_copy(out=oc[:], in_=o_ps[h][:])
                ocs.append(oc)

            # The normalize tail as small closures, popped one per jt in
            # the next chunk so DVE never runs a long chain that delays the
            # PSUM-slot-releasing y-copies (PE priority inversion).
            st8 = {}

            def p_zz():
                st8["zz"] = pool_zz.tile([33, IC], f32, tag="zz", name="zz")
                for h in range(2):
                    nc.vector.tensor_copy(out=st8["zz"][32 * h:32 * h + 1, :],
                                          in_=ocs[h][D:D + 1, :])

            def p_recip(half):
                if half == 0:
                    st8["rz"] = pool_rz.tile([33, IC], f32, tag="rz", name="rz")
                sl = slice(half * (IC // 2), (half + 1) * (IC // 2))
                nc.vector.reciprocal(out=st8["rz"][:, sl], in_=st8["zz"][:, sl])

            def p_bcast():
                st8["rz1"] = pool_rz.tile([1, IC], f32, tag="rz1", name="rz1")
                nc.vector.tensor_copy(out=st8["rz1"][:], in_=st8["rz"][32:33, :])
                st8["rb"] = []
                for h, rzh in ((0, st8["rz"]), (1, st8["rz1"])):
                    rb = pool_rb.tile([D, IC], f32, tag="rb", name=f"rb{h}")
                    nc.gpsimd.partition_broadcast(rb[:], rzh[0:1, :])
                    st8["rb"].append(rb)

            def p_mul(h):
                at = ATc[(b, icx)][D * h:D * (h + 1), :]
                nc.vector.tensor_mul(out=at, in0=ocs[h][0:D, :],
                                     in1=st8["rb"][h][:])
                nc.vector.tensor_scalar_add(
                    out=at, in0=at, scalar1=bv_sb[D * h:D * (h + 1), :])

            return [p_zz, lambda: p_recip(0), lambda: p_recip(1), p_bcast,
                    lambda: p_mul(0), lambda: p_mul(1)]

        # ---- program ----
        for it in proj_items(0):
            it()
        heavy = proj_items(1)
        ready, delay, norm2 = [], [], None
        for b in range(B):
            for icx in range(NQC):
                norm2 = attn_chunk(b, icx, ready, heavy, norm2)
                ready.extend(delay)
                delay = outproj_units(b, icx)
        while heavy:
            heavy.pop(0)()
        for p in norm2:
            p()
        while ready:
            ready.pop(0)()
        for u in delay:
            u()

    nc.compile()
    _cache[key] = nc
    return nc


def _prepare(x, attn_mask, Wq, bq, Wk, bk, Wv, bv, Wo):
    mask = np.asarray(attn_mask).astype(bool)
    xf = np.asarray(x, dtype=np.float32).reshape(B, T, E)

    nkt = []
    cols = []       # packed kv token features, (KP, E) f32
    mbcols = []     # per packed slot: 0 keep / -1e30 pad
    for b in range(B):
        idx = np.nonzero(mask[b])[0]
        nk = len(idx)
        ntiles = max(1, (nk + 127) // 128)
        npad = ntiles * 128
        feats = np.zeros((npad, E), dtype=np.float32)
        feats[:nk] = xf[b, idx, :]
        bias = np.full(npad, -1e30, dtype=np.float32)
        bias[:nk] = 0.0
        nkt.append(ntiles)
        cols.append(feats)
        mbcols.append(bias)

    xk = np.ascontiguousarray(np.concatenate(cols, 0).T).astype(_BF16)
    mb_flat = np.concatenate(mbcols)
    ntt = nkt[0] + nkt[1]
    mb = np.ascontiguousarray(mb_flat.reshape(ntt, 128).T)

    xT = np.ascontiguousarray(xf.reshape(BT, E).T).astype(_BF16)

    in_maps = []
    for c in range(N_CORES):
        sl = slice(c * PCN, (c + 1) * PCN)
        in_maps.append({
            "xT": xT, "xk": xk, "mb": mb,
            "wq": np.ascontiguousarray(Wq[:, sl]).astype(_BF16),
            "wk": np.ascontiguousarray(Wk[:, sl]).astype(_BF16),
            "wv": np.ascontiguousarray(Wv[:, sl]).astype(_BF16),
            "wo": np.ascontiguousarray(Wo[sl, :]).astype(_BF16),
            "bq": np.ascontiguousarray(bq[sl]).reshape(PCN, 1).astype(np.float32),
            "bk": np.ascontiguousarray(bk[sl]).reshape(PCN, 1).astype(np.float32),
            "bv": np.ascontiguousarray(bv[sl]).reshape(PCN, 1).astype(np.float32),
        })
    return nkt, in_maps


def _run(inputs, trace=False, tmpdir=None):
    from concourse.bass_utils import run_bass_kernel_spmd

    nkt, in_maps = _prepare(
        inputs["x"], inputs["attn_mask"], inputs["Wq"], inputs["bq"],
        inputs["Wk"], inputs["bk"], inputs["Wv"], inputs["bv"], inputs["Wo"])
    nc = _build(nkt)
    res = run_bass_kernel_spmd(nc, in_maps, list(range(N_CORES)),
                               trace=trace, tmpdir=tmpdir)
    yT = np.zeros((E, BT), dtype=np.float64)
    for c in range(N_CORES):
        yT += np.asarray(res.results[c]["yT"], dtype=np.float64)
    y = yT.T.astype(np.float32) + inputs["bo"].astype(np.float32)
    return y.reshape(B, T, E), res


def kernel(**inputs):
    y, _ = _run(inputs)
    return y
